# revision 5
# baseline (speedup 1.0000x reference)
"""Trainium2 Bass kernel for nn_CustomLoss: weighted-CE + all-pairs windowed SSIM BCE loss.

Strategy: pure data-parallel over batch B=32 -> 4 videos per core on 8 cores.
Per core, per video (layout: partitions = 64h x 2 channel-stack, free = F,CP,W):
  - 7x7 box filters via TensorE: banded 0/1 H-matrix stationary, 7 PSUM-accumulated
    matmuls over w-shifted rhs slices (x, x^2 per frame; x_i*x_j per pair)
  - all scale/bias constants folded into ScalarE PSUM->SBUF copies
  - SSIM map algebra on DVE (bf16), reciprocal via Abs_reciprocal_sqrt+Square on
    ScalarE (single ACT table set with Copy -> no table reloads)
  - final S = num*r fused with the spatial reduction via scalar_tensor_tensor
    accum_out -> per-pair partial sums [116, 28], DMA'd out per video
Host: sum 116 partials per pair -> ssim means -> BCE; CE from predictions.

Algebra (C1 dropped where rel. contribution < 1e-6):
  U' = sqrt(2*COVN)*ux             (ACT copy scale from PSUM)
  Q  = U'*U' = 2*COVN*ux^2         (DVE, stored in AV slot 0)
  V' = 2*COVN*(uxx-ux^2)+C2        (ACT copy scale+bias C2, minus Q; AV slot 1)
  P2 = U'_i*U'_j = 2*COVN*ux_i*ux_j
  num2 = (2*COVN/49)*Sxy + C2 - P2
  num  = P2*num2
  den12 = AV_i + AV_j  ->  den = den12.q * den12.v = 4*COVN^2*... (den'' )
  S = 4*num/den'' via r = Square(2*rsqrt(den''))
"""

import numpy as np
import ml_dtypes

B, F, C, H, W = 32, 8, 16, 64, 64
NCORES = 8
BSH = B // NCORES          # 4 videos per core
CP = C // 2                # channel pairs stacked on partitions
WIN = 7
HO = H - WIN + 1           # 58
NP_WIN = WIN * WIN
COV_NORM = NP_WIN / (NP_WIN - 1.0)
C1 = 0.01 ** 2
C2 = 0.03 ** 2
NPAIR = F * (F - 1) // 2   # 28
NPART = 2 * HO             # 116 used partitions

_CACHE = {}


def _pair_index(i, j):
    # triu order (row-major), matches np.triu_indices(F, 1)
    base = i * (2 * F - i - 1) // 2
    return base + (j - i - 1)


def _build_program():
    import concourse.bass as bass
    import concourse.bacc as bacc
    import concourse.tile as tile
    from concourse import mybir

    f32 = mybir.dt.float32
    bf16 = mybir.dt.bfloat16
    AF = mybir.ActivationFunctionType
    ALU = mybir.AluOpType

    SQ = float(np.sqrt(2.0 * COV_NORM))     # U' scale on ux
    SXY = 2.0 * COV_NORM / NP_WIN           # Sxy -> 2*COVN*uxy
    # S = 4 * num / den''  -> Square scale 2.0

    nc = bacc.Bacc(None, target_bir_lowering=False)

    feat = nc.dram_tensor([BSH, F, C, H, W], f32, kind="ExternalInput")
    band = nc.dram_tensor([128, NPART], bf16, kind="ExternalInput")
    out = nc.dram_tensor([BSH, NPART, NPAIR], f32, kind="ExternalOutput")

    # element strides of feat
    s_b = F * C * H * W
    s_f = C * H * W
    s_c = H * W

    def ap_of(x):
        return x[:] if not isinstance(x, bass.AP) else x

    def bcast(t_ap, nj):
        # t_ap: AP [128, ...tail] for one frame slot; broadcast a new dim nj
        return bass.AP(
            tensor=t_ap.tensor,
            offset=t_ap.offset,
            ap=[t_ap.ap[0], [0, nj]] + list(t_ap.ap[1:]),
        )

    with tile.TileContext(nc) as tc:
        with (
            tc.tile_pool(name="consts", bufs=1) as consts,
            tc.tile_pool(name="stage", bufs=1) as stage_p,
            tc.tile_pool(name="xp", bufs=2) as xp,
            tc.tile_pool(name="x2p", bufs=1) as x2p,
            tc.tile_pool(name="frameq", bufs=2) as frameq,
            tc.tile_pool(name="frameu", bufs=1) as frameu,
            tc.tile_pool(name="tp", bufs=3) as tp,
            tc.tile_pool(name="uxyp", bufs=2) as uxyp,
            tc.tile_pool(name="math", bufs=8) as mathp,
            tc.tile_pool(name="den12p", bufs=2) as den12p,
            tc.tile_pool(name="sp", bufs=3) as sp,
            tc.tile_pool(name="psA", bufs=2, space="PSUM") as psA,
            tc.tile_pool(name="psB", bufs=5, space="PSUM") as psB,
            tc.tile_pool(name="res", bufs=2) as resp,
        ):
            band_sb = consts.tile([128, NPART], bf16)
            nc.sync.dma_start(out=band_sb[:], in_=band[:])

            for b in range(BSH):
                stg = stage_p.tile([128, F, CP, W], f32, tag="stg")
                for q in range(2):
                    src = ap_of(feat)
                    src_ap = bass.AP(
                        tensor=src.tensor,
                        offset=src.offset + b * s_b + q * s_c,
                        ap=[[W, H], [s_f, F], [2 * s_c, CP], [1, W]],
                    )
                    nc.sync.dma_start(out=stg[64 * q:64 * q + 64, :, :, :], in_=src_ap)

                X = xp.tile([128, F, CP, W], bf16, tag="X")
                nc.scalar.activation(X[:], stg[:], AF.Copy)
                X2 = x2p.tile([128, F, CP, W], bf16, tag="X2")
                nc.vector.tensor_mul(X2[:], X[:], X[:])

                # per-frame filtered stats
                Up = frameq.tile([128, F, CP, HO], bf16, tag="Up")
                AV = frameq.tile([128, F, 2, CP, HO], bf16, tag="AV")
                uxc = frameu.tile([128, F, CP, HO], bf16, tag="uxc")
                for kf in range(F):
                    ps = psA.tile([128, 512], f32, tag="psa")
                    for dw in range(WIN):
                        nc.tensor.matmul(
                            ps[0:NPART, 0:CP * HO],
                            band_sb[:, 0:NPART],
                            X[:, kf, :, dw:dw + HO],
                            start=(dw == 0),
                            stop=(dw == WIN - 1),
                        )
                    nc.scalar.activation(
                        Up[0:NPART, kf, :, :], ps[0:NPART, 0:CP * HO], AF.Copy,
                        scale=SQ / NP_WIN,
                    )
                    ps2 = psA.tile([128, 512], f32, tag="psa")
                    for dw in range(WIN):
                        nc.tensor.matmul(
                            ps2[0:NPART, 0:CP * HO],
                            band_sb[:, 0:NPART],
                            X2[:, kf, :, dw:dw + HO],
                            start=(dw == 0),
                            stop=(dw == WIN - 1),
                        )
                    # uxc = 2*COVN*uxx + C2
                    nc.scalar.activation(
                        uxc[0:NPART, kf, :, :], ps2[0:NPART, 0:CP * HO], AF.Copy,
                        scale=SXY, bias=C2,
                    )
                # Q = U'^2 = 2*COVN*ux^2 -> AV slot 0
                nc.vector.tensor_mul(
                    AV[0:NPART, :, 0, :, :], Up[0:NPART], Up[0:NPART]
                )
                # V' = (2*COVN*uxx + C2) - Q -> AV slot 1
                nc.vector.tensor_sub(
                    AV[0:NPART, :, 1, :, :], uxc[0:NPART], AV[0:NPART, :, 0, :, :]
                )

                pairsum = resp.tile([128, NPAIR], f32, tag="pairsum")

                for i in range(F - 1):
                    nj = F - 1 - i
                    t = tp.tile([128, nj, CP, W], bf16, tag="t")
                    nc.vector.tensor_mul(
                        t[:], bcast(X[:, i, :, :], nj), X[:, i + 1:F, :, :]
                    )

                    # uxy2 = (2*COVN/49)*Sxy + C2 for the group
                    uxy2 = uxyp.tile([128, nj, CP, HO], bf16, tag="uxy2")
                    for j in range(nj):
                        ps = psB.tile([128, 512], f32, tag="psb")
                        for dw in range(WIN):
                            nc.tensor.matmul(
                                ps[0:NPART, 0:CP * HO],
                                band_sb[:, 0:NPART],
                                t[:, j, :, dw:dw + HO],
                                start=(dw == 0),
                                stop=(dw == WIN - 1),
                            )
                        nc.scalar.activation(
                            uxy2[0:NPART, j, :, :], ps[0:NPART, 0:CP * HO], AF.Copy,
                            scale=SXY, bias=C2,
                        )

                    P2 = mathp.tile([128, nj, CP, HO], bf16, tag="mt")
                    nc.vector.tensor_mul(
                        P2[0:NPART], bcast(Up[:, i, :, :], nj)[0:NPART],
                        Up[0:NPART, i + 1:F, :, :],
                    )
                    num2 = mathp.tile([128, nj, CP, HO], bf16, tag="mt")
                    nc.vector.tensor_sub(num2[0:NPART], uxy2[0:NPART], P2[0:NPART])
                    num = mathp.tile([128, nj, CP, HO], bf16, tag="mt")
                    nc.vector.tensor_mul(num[0:NPART], P2[0:NPART], num2[0:NPART])

                    den12 = den12p.tile([128, nj, 2, CP, HO], bf16, tag="mt2")
                    nc.vector.tensor_add(
                        den12[0:NPART],
                        bcast(AV[:, i, :, :, :], nj)[0:NPART],
                        AV[0:NPART, i + 1:F, :, :, :],
                    )
                    den = mathp.tile([128, nj, CP, HO], bf16, tag="mt")
                    nc.vector.tensor_mul(
                        den[0:NPART], den12[0:NPART, :, 0, :, :],
                        den12[0:NPART, :, 1, :, :],
                    )
                    rsq = mathp.tile([128, nj, CP, HO], bf16, tag="mt")
                    nc.scalar.activation(
                        rsq[0:NPART], den[0:NPART], AF.Abs_reciprocal_sqrt
                    )
                    r = mathp.tile([128, nj, CP, HO], bf16, tag="mt")
                    nc.scalar.activation(r[0:NPART], rsq[0:NPART], AF.Square, scale=2.0)

                    # S = num * r, fused with free-dim sum -> pairsum column
                    for j in range(nj):
                        p = _pair_index(i, i + 1 + j)
                        Sout = sp.tile([128, CP * HO], bf16, tag="S")
                        nc.vector.scalar_tensor_tensor(
                            Sout[0:NPART],
                            num[0:NPART, j, :, :],
                            1.0,
                            r[0:NPART, j, :, :],
                            ALU.mult,
                            ALU.mult,
                            accum_out=pairsum[0:NPART, p:p + 1],
                        )

                nc.sync.dma_start(out=out[b, :, :], in_=pairsum[0:NPART, :])

    nc.compile()
    return nc, feat.name, band.name, out.name


def _make_consts():
    band = np.zeros((128, NPART), dtype=np.float32)
    for s in range(2):
        for ho in range(HO):
            band[64 * s + ho:64 * s + ho + WIN, HO * s + ho] = 1.0
    return band.astype(ml_dtypes.bfloat16)


def make_in_maps(features):
    nc, feat_name, band_name, out_name = _CACHE["prog"]
    band = _make_consts()
    feats = np.ascontiguousarray(features, dtype=np.float32)
    return [
        {feat_name: feats[k * BSH:(k + 1) * BSH], band_name: band}
        for k in range(NCORES)
    ]


def kernel(predictions, features, labels):
    from concourse.bass_utils import run_bass_kernel_spmd

    if "prog" not in _CACHE:
        _CACHE["prog"] = _build_program()
    nc, feat_name, band_name, out_name = _CACHE["prog"]

    in_maps = make_in_maps(features)
    res = run_bass_kernel_spmd(nc, in_maps, core_ids=list(range(NCORES)))
    sums = np.concatenate([r[out_name] for r in res.results], axis=0)  # [32,116,28]

    ssim_pair = sums.astype(np.float64).sum(axis=1) / (C * HO * HO)  # [32, 28]

    labels = np.asarray(labels).astype(np.int64)
    preds = np.asarray(predictions).astype(np.float64)

    # weighted CE (torch CrossEntropyLoss with weights [10, 1])
    mx = preds.max(axis=1, keepdims=True)
    logp = preds - mx - np.log(np.exp(preds - mx).sum(axis=1, keepdims=True))
    nll = -logp[np.arange(B), labels]
    wts = np.where(labels == 0, 10.0, 1.0)
    cce = (wts * nll).sum() / wts.sum()

    # BCE on mean pair-similarity
    sim = np.clip(ssim_pair + 0.5, 0.0, 1.0)
    avg_sim = sim.mean(axis=1)
    t = (labels == 0).astype(np.float64)
    log_p = np.maximum(np.log(np.maximum(avg_sim, 1e-300)), -100.0)
    log_1mp = np.maximum(np.log(np.maximum(1.0 - avg_sim, 1e-300)), -100.0)
    bce = -(t * log_p + (1.0 - t) * log_1mp)
    inconsistency = bce.mean()

    return np.float32(cce + 4.0 * inconsistency)


# revision 6
# speedup vs baseline: 1.1205x; 1.1205x over previous
"""Trainium2 Bass kernel for nn_CustomLoss: weighted-CE + all-pairs windowed SSIM BCE loss.

Strategy: pure data-parallel over batch B=32 -> 4 videos per core on 8 cores.
Per core, per video (layout: partitions = 64h x 2 channel-stack, free = F,CP,W):
  - 7x7 box filters via TensorE: banded 0/1 H-matrix stationary, 7 PSUM-accumulated
    matmuls over w-shifted rhs slices (x, x^2 per frame; x_i*x_j per pair)
  - all scale/bias constants folded into ScalarE PSUM->SBUF copies
  - SSIM map algebra on DVE (bf16), reciprocal via Abs_reciprocal_sqrt+Square on
    ScalarE (single ACT table set with Copy -> no table reloads)
  - final S = num*r fused with the spatial reduction via scalar_tensor_tensor
    accum_out -> per-pair partial sums [116, 28], DMA'd out per video
Host: sum 116 partials per pair -> ssim means -> BCE; CE from predictions.

Algebra (C1 dropped where rel. contribution < 1e-6):
  U' = sqrt(2*COVN)*ux             (ACT copy scale from PSUM)
  Q  = U'*U' = 2*COVN*ux^2         (DVE, stored in AV slot 0)
  V' = 2*COVN*(uxx-ux^2)+C2        (ACT copy scale+bias C2, minus Q; AV slot 1)
  P2 = U'_i*U'_j = 2*COVN*ux_i*ux_j
  num2 = (2*COVN/49)*Sxy + C2 - P2
  num  = P2*num2
  den12 = AV_i + AV_j  ->  den = den12.q * den12.v = 4*COVN^2*... (den'' )
  S = 4*num/den'' via r = Square(2*rsqrt(den''))
"""

import numpy as np
import ml_dtypes

B, F, C, H, W = 32, 8, 16, 64, 64
NCORES = 8
BSH = B // NCORES          # 4 videos per core
CP = C // 2                # channel pairs stacked on partitions
WIN = 7
HO = H - WIN + 1           # 58
NP_WIN = WIN * WIN
COV_NORM = NP_WIN / (NP_WIN - 1.0)
C1 = 0.01 ** 2
C2 = 0.03 ** 2
NPAIR = F * (F - 1) // 2   # 28
NPART = 2 * HO             # 116 used partitions

_CACHE = {}


def _pair_index(i, j):
    # triu order (row-major), matches np.triu_indices(F, 1)
    base = i * (2 * F - i - 1) // 2
    return base + (j - i - 1)


def _build_program():
    import concourse.bass as bass
    import concourse.bacc as bacc
    import concourse.tile as tile
    from concourse import mybir

    f32 = mybir.dt.float32
    bf16 = mybir.dt.bfloat16
    AF = mybir.ActivationFunctionType
    ALU = mybir.AluOpType

    SQ = float(np.sqrt(2.0 * COV_NORM))     # U' scale on ux
    SXY = 2.0 * COV_NORM / NP_WIN           # Sxy -> 2*COVN*uxy
    # S = 4 * num / den''  -> Square scale 2.0

    nc = bacc.Bacc(None, target_bir_lowering=False)

    feat = nc.dram_tensor([BSH, F, C, H, W], f32, kind="ExternalInput")
    band = nc.dram_tensor([128, NPART], bf16, kind="ExternalInput")
    out = nc.dram_tensor([BSH, NPART, NPAIR], f32, kind="ExternalOutput")

    # element strides of feat
    s_b = F * C * H * W
    s_f = C * H * W
    s_c = H * W

    def ap_of(x):
        return x[:] if not isinstance(x, bass.AP) else x

    def bcast(t_ap, nj):
        # t_ap: AP [128, ...tail] for one frame slot; broadcast a new dim nj
        return bass.AP(
            tensor=t_ap.tensor,
            offset=t_ap.offset,
            ap=[t_ap.ap[0], [0, nj]] + list(t_ap.ap[1:]),
        )

    with tile.TileContext(nc) as tc:
        with (
            tc.tile_pool(name="consts", bufs=1) as consts,
            tc.tile_pool(name="stage", bufs=1) as stage_p,
            tc.tile_pool(name="xp", bufs=2) as xp,
            tc.tile_pool(name="x2p", bufs=1) as x2p,
            tc.tile_pool(name="frameq", bufs=2) as frameq,
            tc.tile_pool(name="frameu", bufs=1) as frameu,
            tc.tile_pool(name="tp", bufs=3) as tp,
            tc.tile_pool(name="uxyp", bufs=2) as uxyp,
            tc.tile_pool(name="math", bufs=8) as mathp,
            tc.tile_pool(name="den12p", bufs=2) as den12p,
            tc.tile_pool(name="sp", bufs=3) as sp,
            tc.tile_pool(name="psA", bufs=3, space="PSUM") as psA,
            tc.tile_pool(name="psB", bufs=5, space="PSUM") as psB,
            tc.tile_pool(name="res", bufs=2) as resp,
        ):
            band_sb = consts.tile([128, NPART], bf16)
            nc.sync.dma_start(out=band_sb[:], in_=band[:])

            for b in range(BSH):
                stg = stage_p.tile([128, F, CP, W], f32, tag="stg")
                for q in range(2):
                    src = ap_of(feat)
                    src_ap = bass.AP(
                        tensor=src.tensor,
                        offset=src.offset + b * s_b + q * s_c,
                        ap=[[W, H], [s_f, F], [2 * s_c, CP], [1, W]],
                    )
                    nc.sync.dma_start(out=stg[64 * q:64 * q + 64, :, :, :], in_=src_ap)

                X = xp.tile([128, F, CP, W], bf16, tag="X")
                nc.scalar.activation(X[:], stg[:], AF.Copy)
                X2 = x2p.tile([128, F, CP, W], bf16, tag="X2")
                nc.vector.tensor_mul(X2[:], X[:], X[:])

                # per-frame filtered stats
                Up = frameq.tile([128, F, CP, HO], bf16, tag="Up")
                AV = frameq.tile([128, F, 2, CP, HO], bf16, tag="AV")
                uxc = frameu.tile([128, F, CP, HO], bf16, tag="uxc")
                for kf in range(F):
                    ps = psA.tile([128, 512], f32, tag="psa")
                    for dw in range(WIN):
                        nc.tensor.matmul(
                            ps[0:NPART, 0:CP * HO],
                            band_sb[:, 0:NPART],
                            X[:, kf, :, dw:dw + HO],
                            start=(dw == 0),
                            stop=(dw == WIN - 1),
                        )
                    nc.scalar.activation(
                        Up[0:NPART, kf, :, :], ps[0:NPART, 0:CP * HO], AF.Copy,
                        scale=SQ / NP_WIN,
                    )
                    ps2 = psA.tile([128, 512], f32, tag="psa")
                    for dw in range(WIN):
                        nc.tensor.matmul(
                            ps2[0:NPART, 0:CP * HO],
                            band_sb[:, 0:NPART],
                            X2[:, kf, :, dw:dw + HO],
                            start=(dw == 0),
                            stop=(dw == WIN - 1),
                        )
                    # uxc = 2*COVN*uxx + C2
                    nc.scalar.activation(
                        uxc[0:NPART, kf, :, :], ps2[0:NPART, 0:CP * HO], AF.Copy,
                        scale=SXY, bias=C2,
                    )
                # Q = U'^2 = 2*COVN*ux^2 -> AV slot 0
                nc.vector.tensor_mul(
                    AV[0:NPART, :, 0, :, :], Up[0:NPART], Up[0:NPART]
                )
                # V' = (2*COVN*uxx + C2) - Q -> AV slot 1
                nc.vector.tensor_sub(
                    AV[0:NPART, :, 1, :, :], uxc[0:NPART], AV[0:NPART, :, 0, :, :]
                )

                pairsum = resp.tile([128, NPAIR], f32, tag="pairsum")

                for i in range(F - 1):
                    nj = F - 1 - i
                    t = tp.tile([128, nj, CP, W], bf16, tag="t")
                    nc.vector.tensor_mul(
                        t[:], bcast(X[:, i, :, :], nj), X[:, i + 1:F, :, :]
                    )

                    # uxy2 = (2*COVN/49)*Sxy + C2 for the group
                    uxy2 = uxyp.tile([128, nj, CP, HO], bf16, tag="uxy2")
                    for j in range(nj):
                        ps = psB.tile([128, 512], f32, tag="psb")
                        for dw in range(WIN):
                            nc.tensor.matmul(
                                ps[0:NPART, 0:CP * HO],
                                band_sb[:, 0:NPART],
                                t[:, j, :, dw:dw + HO],
                                start=(dw == 0),
                                stop=(dw == WIN - 1),
                            )
                        nc.scalar.activation(
                            uxy2[0:NPART, j, :, :], ps[0:NPART, 0:CP * HO], AF.Copy,
                            scale=SXY, bias=C2,
                        )

                    P2 = mathp.tile([128, nj, CP, HO], bf16, tag="mt")
                    nc.vector.tensor_mul(
                        P2[0:NPART], bcast(Up[:, i, :, :], nj)[0:NPART],
                        Up[0:NPART, i + 1:F, :, :],
                    )
                    num2 = mathp.tile([128, nj, CP, HO], bf16, tag="mt")
                    nc.vector.tensor_sub(num2[0:NPART], uxy2[0:NPART], P2[0:NPART])
                    num = mathp.tile([128, nj, CP, HO], bf16, tag="mt")
                    nc.vector.tensor_mul(num[0:NPART], P2[0:NPART], num2[0:NPART])

                    den12 = den12p.tile([128, nj, 2, CP, HO], bf16, tag="mt2")
                    nc.vector.tensor_add(
                        den12[0:NPART],
                        bcast(AV[:, i, :, :, :], nj)[0:NPART],
                        AV[0:NPART, i + 1:F, :, :, :],
                    )
                    den = mathp.tile([128, nj, CP, HO], bf16, tag="mt")
                    nc.vector.tensor_mul(
                        den[0:NPART], den12[0:NPART, :, 0, :, :],
                        den12[0:NPART, :, 1, :, :],
                    )
                    rsq = mathp.tile([128, nj, CP, HO], bf16, tag="mt")
                    nc.scalar.activation(
                        rsq[0:NPART], den[0:NPART], AF.Abs_reciprocal_sqrt
                    )
                    r = mathp.tile([128, nj, CP, HO], bf16, tag="mt")
                    nc.scalar.activation(r[0:NPART], rsq[0:NPART], AF.Square, scale=2.0)

                    # S = num * r, fused with free-dim sum -> pairsum column
                    for j in range(nj):
                        p = _pair_index(i, i + 1 + j)
                        Sout = sp.tile([128, CP * HO], bf16, tag="S")
                        nc.vector.scalar_tensor_tensor(
                            Sout[0:NPART],
                            num[0:NPART, j, :, :],
                            1.0,
                            r[0:NPART, j, :, :],
                            ALU.mult,
                            ALU.mult,
                            accum_out=pairsum[0:NPART, p:p + 1],
                        )

                nc.sync.dma_start(out=out[b, :, :], in_=pairsum[0:NPART, :])

    nc.compile()
    return nc, feat.name, band.name, out.name


def _make_consts():
    band = np.zeros((128, NPART), dtype=np.float32)
    for s in range(2):
        for ho in range(HO):
            band[64 * s + ho:64 * s + ho + WIN, HO * s + ho] = 1.0
    return band.astype(ml_dtypes.bfloat16)


def make_in_maps(features):
    nc, feat_name, band_name, out_name = _CACHE["prog"]
    band = _make_consts()
    feats = np.ascontiguousarray(features, dtype=np.float32)
    return [
        {feat_name: feats[k * BSH:(k + 1) * BSH], band_name: band}
        for k in range(NCORES)
    ]


def kernel(predictions, features, labels):
    from concourse.bass_utils import run_bass_kernel_spmd

    if "prog" not in _CACHE:
        _CACHE["prog"] = _build_program()
    nc, feat_name, band_name, out_name = _CACHE["prog"]

    in_maps = make_in_maps(features)
    res = run_bass_kernel_spmd(nc, in_maps, core_ids=list(range(NCORES)))
    sums = np.concatenate([r[out_name] for r in res.results], axis=0)  # [32,116,28]

    ssim_pair = sums.astype(np.float64).sum(axis=1) / (C * HO * HO)  # [32, 28]

    labels = np.asarray(labels).astype(np.int64)
    preds = np.asarray(predictions).astype(np.float64)

    # weighted CE (torch CrossEntropyLoss with weights [10, 1])
    mx = preds.max(axis=1, keepdims=True)
    logp = preds - mx - np.log(np.exp(preds - mx).sum(axis=1, keepdims=True))
    nll = -logp[np.arange(B), labels]
    wts = np.where(labels == 0, 10.0, 1.0)
    cce = (wts * nll).sum() / wts.sum()

    # BCE on mean pair-similarity
    sim = np.clip(ssim_pair + 0.5, 0.0, 1.0)
    avg_sim = sim.mean(axis=1)
    t = (labels == 0).astype(np.float64)
    log_p = np.maximum(np.log(np.maximum(avg_sim, 1e-300)), -100.0)
    log_1mp = np.maximum(np.log(np.maximum(1.0 - avg_sim, 1e-300)), -100.0)
    bce = -(t * log_p + (1.0 - t) * log_1mp)
    inconsistency = bce.mean()

    return np.float32(cce + 4.0 * inconsistency)
